# revision 4
# baseline (speedup 1.0000x reference)
"""HGCN (2x hyperbolic GCN layer + MLP head) as a distributed Bass/Tile kernel
for 8 trn2 NeuronCores — ap_gather edition.

Math: logmap0(expmap0(v)) == v for this problem's value ranges, so the network
collapses to
    t2  = sigmoid(meanagg(X) @ W1 + b1)
    t3  = sigmoid(meanagg(t2) @ W2 + b2)
    out = relu(t3 @ W3 + b3) @ W4 + b4
(validated to 6e-7 against the jax reference by the previous baseline; the
numeric pipeline here reproduces the baseline's 0.93% rel err in host sim).

Distribution/layout: dst nodes sharded 8 ways (12500/core). The full node
table lives in SBUF in "octant" layout: partition 16o+j holds features
4j..4j+3 of octant-o nodes (octant o = core o's dst shard, in core o's
processing order), with row 0 zeroed. Each gpsimd Q7 core o gathers, via one
ap_gather custom op per block, the source rows of the edges whose src lies in
octant o (per-dst slot lists padded to W = max octant degree; -1 pads gather
the zero row). A strided DVE tree-sum reduces slots, 3 partition-halving adds
combine the 8 octant partials, and 4 matmuls against host-sliced W[r::4,:]
apply the layer weight while converting the packed [16,dst,4] layout to
[64,dst] — no transposes anywhere. Layer-1 outputs are folded back to packed
layout with 4 one-hot matmuls and AllGathered: the collective output IS the
layer-2 table. Weights are tiny and replicated.
"""

import os
import numpy as np
import ml_dtypes

import concourse.bass as bass
import concourse.bacc as bacc
import concourse.tile as tile
from concourse import mybir
from concourse.library_config import ap_gather as _apg_lib

NC = 8
SH = 12500
D = 64
NDCAP = 384       # dsts per block
POSCAP = 1536     # gather positions per block

BF16 = mybir.dt.bfloat16
F32 = mybir.dt.float32
I16 = mybir.dt.int16

bfloat16 = ml_dtypes.bfloat16


def _ceil16(x):
    return (x + 15) // 16 * 16


def _ceil64(x):
    # 64-position (4 idx-column) granularity: the ap_gather ucode miswrites
    # positions 16..31 of each 128-chunk when its idx AP starts at a column
    # offset = 3 (mod 4), so keep every block's idx slice 4-column aligned
    return (x + 63) // 64 * 64


def _preprocess(edge_index, n_nodes):
    """Host-side index preprocessing (layout only, no input arithmetic)."""
    src = np.asarray(edge_index[0], np.int64)
    dst = np.asarray(edge_index[1], np.int64)
    core = dst // SH
    octv = src // SH

    pm = []
    pos_of = np.empty(n_nodes, np.int64)
    for k in range(NC):
        m = core == k
        d = dst[m] - k * SH
        o = octv[m]
        s = src[m]
        deg = np.bincount(d, minlength=SH)
        dego = np.bincount(d * NC + o, minlength=SH * NC).reshape(SH, NC)
        W = np.maximum(dego.max(axis=1), 1)
        order = np.argsort(-W, kind="stable")
        pos_of[k * SH + order] = np.arange(SH)
        pm.append(dict(d=d, o=o, s=s, deg=deg, W=W, order=order))

    # uniform W envelope across cores (one compiled program for all cores)
    Wu = np.max(np.stack([p["W"][p["order"]] for p in pm]), axis=0)

    blocks = []
    p0 = 0
    i = 0
    while i < SH:
        Wv = int(Wu[i])
        j = i
        while j < SH and Wu[j] == Wv:
            j += 1
        a = i
        while a < j:
            nd = min(NDCAP, j - a, POSCAP // Wv)
            b_ = a + nd
            plen = _ceil64(nd * Wv)
            blocks.append((a, b_, Wv, p0, plen))
            p0 += plen
            a = b_
        i = j
    TOT = p0

    pos_base = np.empty(SH, np.int64)
    for (a, b_, Wv, q0, plen) in blocks:
        pos_base[a:b_] = q0 + np.arange(b_ - a) * Wv

    idxw = np.empty((NC, 128, TOT // 16), np.int16)
    dinv = np.empty((NC, 16, SH), np.float32)
    for k in range(NC):
        p = pm[k]
        d, o, s = p["d"], p["o"], p["s"]
        order = p["order"]
        inv = np.empty(SH, np.int64)
        inv[order] = np.arange(SH)
        key = d * NC + o
        ordE = np.argsort(key, kind="stable")
        ke = key[ordE]
        first = np.r_[True, ke[1:] != ke[:-1]]
        starts = np.flatnonzero(first)
        gid = np.cumsum(first) - 1
        rank = np.arange(len(ke)) - starts[gid]
        pe = pos_base[inv[d[ordE]]] + rank
        val = (pos_of[s[ordE]] + 1).astype(np.int16)
        L = np.full((NC, TOT), 0, np.int16)   # pads gather zero row 0
        L[o[ordE], pe] = val
        for oo in range(NC):
            idxw[k, oo * 16:(oo + 1) * 16, :] = L[oo].reshape(TOT // 16, 16).T
        dv = (1.0 / np.maximum(p["deg"][order], 1)).astype(np.float32)
        dinv[k] = np.broadcast_to(dv, (16, SH))
    orders = np.stack([p["order"] for p in pm])
    return dict(blocks=blocks, TOT=TOT, idxw=idxw, dinv=dinv, orders=orders)


def _build_program(blocks, TOT, dbg=False):
    nc = bacc.Bacc("TRN2", target_bir_lowering=False, debug=False,
                   enable_asserts=False, num_devices=NC)
    ADD = mybir.AluOpType.add
    MULT = mybir.AluOpType.mult

    xtab_d = nc.dram_tensor("xtab", [128, SH + 1, 4], BF16,
                            kind="ExternalInput")
    idx_d = nc.dram_tensor("idx", [128, TOT // 16], I16, kind="ExternalInput")
    dinv_d = nc.dram_tensor("dinv", [D, SH], BF16, kind="ExternalInput")
    w1_d = nc.dram_tensor("w1s", [128, 4 * D], BF16, kind="ExternalInput")
    w2_d = nc.dram_tensor("w2s", [128, 4 * D], BF16, kind="ExternalInput")
    w3_d = nc.dram_tensor("w3", [D, 128], BF16, kind="ExternalInput")
    w4_d = nc.dram_tensor("w4", [128, 40], BF16, kind="ExternalInput")
    er_d = nc.dram_tensor("er", [D, 4 * 16], BF16, kind="ExternalInput")
    b1_d = nc.dram_tensor("b1", [D, 1], F32, kind="ExternalInput")
    b2_d = nc.dram_tensor("b2", [D, 1], F32, kind="ExternalInput")
    b3_d = nc.dram_tensor("b3", [128, 1], F32, kind="ExternalInput")
    b4_d = nc.dram_tensor("b4", [40, 1], F32, kind="ExternalInput")
    outT_d = nc.dram_tensor("outT", [40, SH], F32, kind="ExternalOutput")
    t2self = nc.dram_tensor("t2self", [16, SH, 4], BF16)
    t2full = nc.dram_tensor("t2full", [128, SH, 4], BF16)
    dbg_t = {}
    if dbg:
        dbg_a = int(os.environ.get('DBG_A', str(blocks[0][0])))
        (a0, b0, W0, q0_, p0_) = [b for b in blocks if b[0] == dbg_a][0]
        nd0 = b0 - a0
        dbg_t["dG"] = nc.dram_tensor("dG", [128, p0_, 4], BF16,
                                     kind="ExternalOutput")
        dbg_t["dTb"] = nc.dram_tensor("dTb", [128, nd0, 4], BF16,
                                      kind="ExternalOutput")
        dbg_t["dAb"] = nc.dram_tensor("dAb", [128, nd0, 4], BF16,
                                      kind="ExternalOutput")
        dbg_t["dst"] = nc.dram_tensor("dst1", [D, nd0], BF16,
                                      kind="ExternalOutput")
        dbg_t["dt2b"] = nc.dram_tensor("dt2b", [16, nd0, 4], BF16,
                                       kind="ExternalOutput")
        dbg_t["dt2full"] = nc.dram_tensor("dt2full", [128, SH, 4], BF16,
                                          kind="ExternalOutput")

    from contextlib import ExitStack
    with tile.TileContext(nc) as tc, ExitStack() as es:
        const = es.enter_context(tc.tile_pool(name="const", bufs=1))
        tabp = es.enter_context(tc.tile_pool(name="tabp", bufs=1))
        gpool = es.enter_context(tc.tile_pool(name="gpool", bufs=2))
        tpool = es.enter_context(tc.tile_pool(name="tpool", bufs=1))
        spool = es.enter_context(tc.tile_pool(name="spool", bufs=2))
        psum = es.enter_context(tc.tile_pool(name="psum", bufs=2,
                                             space="PSUM"))

        nc.gpsimd.load_library(_apg_lib)
        idx_s = const.tile([128, TOT // 16], I16)
        nc.sync.dma_start(out=idx_s[:], in_=idx_d[:])
        dinv_s = const.tile([D, SH], BF16)
        nc.sync.dma_start(out=dinv_s[:], in_=dinv_d[:])
        w1_s = const.tile([128, 4 * D], BF16)
        nc.sync.dma_start(out=w1_s[:], in_=w1_d[:])
        w2_s = const.tile([128, 4 * D], BF16)
        nc.sync.dma_start(out=w2_s[:], in_=w2_d[:])
        w3_s = const.tile([D, 128], BF16)
        nc.sync.dma_start(out=w3_s[:], in_=w3_d[:])
        w4_s = const.tile([128, 40], BF16)
        nc.sync.dma_start(out=w4_s[:], in_=w4_d[:])
        er_s = const.tile([D, 4 * 16], BF16)
        nc.sync.dma_start(out=er_s[:], in_=er_d[:])
        b1_s = const.tile([D, 1], F32)
        nc.sync.dma_start(out=b1_s[:], in_=b1_d[:])
        b2_s = const.tile([D, 1], F32)
        nc.sync.dma_start(out=b2_s[:], in_=b2_d[:])
        b3_s = const.tile([128, 1], F32)
        nc.sync.dma_start(out=b3_s[:], in_=b3_d[:])
        b4_s = const.tile([40, 1], F32)
        nc.sync.dma_start(out=b4_s[:], in_=b4_d[:])

        def do_block(blk, tab, second):
            (a, b_, Wv, q0, plen) = blk
            nd = b_ - a
            dump = dbg and not second and a == int(os.environ.get('DBG_A', str(blocks[0][0])))
            G = gpool.tile([128, POSCAP, 4], BF16, tag="G")
            nc.gpsimd.ap_gather(G[:, :plen, :], tab[:],
                                idx_s[:, q0 // 16:(q0 + plen) // 16],
                                channels=128, num_elems=SH + 1, d=4,
                                num_idxs=plen)
            g4 = G[:, :nd * Wv, :].rearrange("p (n w) f -> p n w f", w=Wv)
            # per-octant slot reduction -> Tb [128, nd, 4] bf16 (one bf16
            # rounding of the f32-exact partial; octant combine happens in
            # the PE's f32 accumulation via the 8x-replicated W slices)
            if Wv == 1:
                red = g4[:, :, 0, :]
            elif Wv == 2:
                Tb = spool.tile([128, NDCAP, 4], BF16, tag="Tb")
                nc.vector.tensor_tensor(out=Tb[:, :nd, :],
                                        in0=g4[:, :, 0, :],
                                        in1=g4[:, :, 1, :], op=ADD)
                red = Tb[:, :nd, :]
            else:
                h = Wv // 2
                ns = h + (Wv % 2)
                T = tpool.tile([128, 1024, 4], F32, tag="T")
                assert nd * ns <= 1024
                t4 = T[:, :nd * ns, :].rearrange("p (n w) f -> p n w f", w=ns)
                nc.vector.tensor_tensor(out=t4[:, :, 0:h, :],
                                        in0=g4[:, :, 0:h, :],
                                        in1=g4[:, :, h:2 * h, :], op=ADD)
                if Wv % 2:
                    nc.vector.tensor_copy(out=t4[:, :, h, :],
                                          in_=g4[:, :, 2 * h, :])
                n = ns
                while n > 2:
                    hh = n // 2
                    nc.vector.tensor_tensor(out=t4[:, :, 0:hh, :],
                                            in0=t4[:, :, 0:hh, :],
                                            in1=t4[:, :, hh:2 * hh, :],
                                            op=ADD)
                    if n % 2:
                        nc.vector.tensor_copy(out=t4[:, :, hh, :],
                                              in_=t4[:, :, n - 1, :])
                    n = hh + (n % 2)
                Tb = spool.tile([128, NDCAP, 4], BF16, tag="Tb")
                nc.vector.tensor_tensor(out=Tb[:, :nd, :],
                                        in0=t4[:, :, 0, :],
                                        in1=t4[:, :, 1, :], op=ADD)
                red = Tb[:, :nd, :]
            ws = w2_s if second else w1_s
            bs = b2_s if second else b1_s
            pmm = psum.tile([D, NDCAP], F32, tag="pm", space="PSUM")
            for r in range(4):
                nc.tensor.matmul(pmm[:, :nd], lhsT=ws[:, r * D:(r + 1) * D],
                                 rhs=red[:, :, r], start=(r == 0),
                                 stop=(r == 3))
            # mean normalization applied post-matmul (linear), one [64,nd] op
            sp = spool.tile([D, NDCAP], F32, tag="sp")
            nc.vector.tensor_tensor(out=sp[:, :nd], in0=pmm[:, :nd],
                                    in1=dinv_s[:, a:b_], op=MULT)
            st = spool.tile([D, NDCAP], BF16, tag="st")
            nc.scalar.activation(st[:, :nd], sp[:, :nd],
                                 mybir.ActivationFunctionType.Sigmoid,
                                 bias=bs[:, :1])
            return (blk, st)

        def do_back(pend, second):
            (blk, st) = pend
            (a, b_, Wv, q0, plen) = blk
            nd = b_ - a
            dump = False
            if not second:
                t2b = spool.tile([16, NDCAP, 4], BF16, tag="t2b")
                for r in range(4):
                    pf = psum.tile([16, NDCAP], F32, tag="pf", space="PSUM")
                    nc.tensor.matmul(pf[:, :nd],
                                     lhsT=er_s[:, r * 16:(r + 1) * 16],
                                     rhs=st[:, :nd], start=True, stop=True)
                    nc.vector.tensor_copy(out=t2b[:, :nd, r], in_=pf[:, :nd])
                nc.sync.dma_start(out=t2self[:, a:b_, :], in_=t2b[:, :nd, :])
            else:
                p3 = psum.tile([128, NDCAP], F32, tag="p3", space="PSUM")
                nc.tensor.matmul(p3[:, :nd], lhsT=w3_s[:], rhs=st[:, :nd],
                                 start=True, stop=True)
                hb = spool.tile([128, NDCAP], BF16, tag="hb")
                nc.scalar.activation(hb[:, :nd], p3[:, :nd],
                                     mybir.ActivationFunctionType.Relu,
                                     bias=b3_s[:, :1])
                p4 = psum.tile([40, NDCAP], F32, tag="p4", space="PSUM")
                nc.tensor.matmul(p4[:, :nd], lhsT=w4_s[:], rhs=hb[:, :nd],
                                 start=True, stop=True)
                o4 = spool.tile([40, NDCAP], F32, tag="o4")
                nc.vector.tensor_scalar_add(o4[:, :nd], p4[:, :nd],
                                            b4_s[:, :1])
                nc.sync.dma_start(out=outT_d[:, a:b_], in_=o4[:, :nd])

        tab = tabp.tile([128, SH + 1, 4], BF16, tag="tab")
        nc.sync.dma_start(out=tab[:], in_=xtab_d[:])
        if dbg and os.environ.get("DBG_MODE", "") == "gatheronly":
            dga = nc.dram_tensor("dGall", [128, TOT, 4], BF16,
                                 kind="ExternalOutput")
            gopool = es.enter_context(tc.tile_pool(
                name="gopool", bufs=int(os.environ.get("GONLY_BUFS", "2"))))
            for (a, b_, Wv, q0, plen) in blocks:
                G = gopool.tile([128, POSCAP, 4], BF16, tag="G")
                nc.gpsimd.ap_gather(G[:, :plen, :], tab[:],
                                    idx_s[:, q0 // 16:(q0 + plen) // 16],
                                    channels=128, num_elems=SH + 1, d=4,
                                    num_idxs=plen)
                nc.sync.dma_start(out=dga[:, q0:q0 + plen, :],
                                  in_=G[:, :plen, :])
            zt = const.tile([40, SH], F32)
            nc.vector.memset(zt[:], 0.0)
            nc.sync.dma_start(out=outT_d[:], in_=zt[:])
        else:
            pend = None
            for blk in blocks:
                new = do_block(blk, tab, second=False)
                if pend is not None:
                    do_back(pend, False)
                pend = new
            do_back(pend, False)
            nc.gpsimd.collective_compute(
                "AllGather",
                mybir.AluOpType.bypass,
                replica_groups=[list(range(NC))],
                ins=[t2self.ap().opt()],
                outs=[t2full.ap().opt()],
            )
            if dbg:
                nc.sync.dma_start(out=dbg_t["dt2full"][:], in_=t2full[:])
            tab2 = tabp.tile([128, SH + 1, 4], BF16, tag="tab")
            nc.sync.dma_start(out=tab2[:, 1:SH + 1, :], in_=t2full[:])
            nc.vector.memset(tab2[:, 0:1, :], 0.0)
            pend = None
            for blk in blocks:
                new = do_block(blk, tab2, second=True)
                if pend is not None:
                    do_back(pend, True)
                pend = new
            do_back(pend, True)

    nc.compile()
    return nc


def _host_inputs(inputs, meta):
    X = np.asarray(inputs["features"], np.float32)
    orders = meta["orders"]
    xtab = np.zeros((128, SH + 1, 4), bfloat16)
    for o in range(NC):
        arr = X[o * SH + orders[o]].astype(bfloat16)       # [SH, 64]
        xtab[16 * o:16 * (o + 1), 1:, :] = (
            arr.reshape(SH, 16, 4).transpose(1, 0, 2))

    def wslices(Wm):
        Wb = np.asarray(Wm, np.float32).astype(bfloat16)
        s = np.concatenate([Wb[r::4, :] for r in range(4)], axis=1)
        return np.tile(s, (8, 1))              # replicate across octant groups

    er = np.zeros((D, 64), bfloat16)
    for j in range(16):
        for r in range(4):
            er[4 * j + r, r * 16 + j] = 1

    common = dict(
        xtab=xtab,
        w1s=wslices(inputs["W1"]),
        w2s=wslices(inputs["W2"]),
        w3=np.asarray(inputs["W3"], np.float32).astype(bfloat16),
        w4=np.asarray(inputs["W4"], np.float32).astype(bfloat16),
        er=er,
        b1=np.asarray(inputs["b1"], np.float32).reshape(D, 1),
        b2=np.asarray(inputs["b2"], np.float32).reshape(D, 1),
        b3=np.asarray(inputs["b3"], np.float32).reshape(128, 1),
        b4=np.asarray(inputs["b4"], np.float32).reshape(40, 1),
    )
    return [dict(common, idx=meta["idxw"][k],
                 dinv=np.broadcast_to(meta["dinv"][k][0], (D, SH))
                 .astype(bfloat16)) for k in range(NC)]


def kernel(features, edge_index, W1, b1, W2, b2, W3, b3, W4, b4):
    n_nodes = features.shape[0]
    inputs = dict(features=features, edge_index=edge_index, W1=W1, b1=b1,
                  W2=W2, b2=b2, W3=W3, b3=b3, W4=W4, b4=b4)
    meta = _preprocess(edge_index, n_nodes)
    nc = _build_program(meta["blocks"], meta["TOT"])
    in_maps = _host_inputs(inputs, meta)
    results = _run_spmd_timed(nc, in_maps,
                              reps=int(os.environ.get("KERNEL_REPS", "0")))
    out = np.empty((n_nodes, 40), np.float32)
    for k in range(NC):
        outT = np.asarray(results[k]["outT"], np.float32)
        out[k * SH + meta["orders"][k]] = outT.T
    return out


def _run_spmd_timed(nc, in_maps, reps=0):
    """bass2jax.run_bass_via_pjrt's multi-core path with inputs device_put
    once (sharded), optional repeated timed executions, and optional NTFF
    profiling (KERNEL_NTFF=1) for the true on-device execution time."""
    import time
    import jax
    from jax.sharding import Mesh, PartitionSpec, NamedSharding
    from jax.experimental.shard_map import shard_map
    from concourse import bass2jax, mybir as mb

    bass2jax.install_neuronx_cc_hook()
    n_cores = len(in_maps)
    partition_name = (nc.partition_id_tensor.name
                      if nc.partition_id_tensor else None)
    in_names, out_names, out_avals, zero_outs = [], [], [], []
    for alloc in nc.m.functions[0].allocations:
        if not isinstance(alloc, mb.MemoryLocationSet):
            continue
        name = alloc.memorylocations[0].name
        if alloc.kind == "ExternalInput":
            if name != partition_name:
                in_names.append(name)
        elif alloc.kind == "ExternalOutput":
            shape = tuple(alloc.tensor_shape)
            dtype = mb.dt.np(alloc.dtype)
            out_avals.append(jax.core.ShapedArray(shape, dtype))
            zero_outs.append(np.zeros(shape, dtype))
            out_names.append(name)
    n_params = len(in_names)
    n_outs = len(out_avals)
    all_in_names = list(in_names) + list(out_names)
    if partition_name is not None:
        all_in_names.append(partition_name)

    def _body(*args):
        operands = list(args)
        if partition_name is not None:
            operands.append(bass2jax.partition_id_tensor())
        return tuple(bass2jax._bass_exec_p.bind(
            *operands, out_avals=tuple(out_avals),
            in_names=tuple(all_in_names), out_names=tuple(out_names),
            lowering_input_output_aliases=(),
            sim_require_finite=True, sim_require_nnan=True, nc=nc))

    devices = jax.devices()[:n_cores]
    mesh = Mesh(np.asarray(devices), ("core",))
    spec = NamedSharding(mesh, PartitionSpec("core"))
    sharded = jax.jit(
        shard_map(_body, mesh=mesh,
                  in_specs=(PartitionSpec("core"),) * (n_params + n_outs),
                  out_specs=(PartitionSpec("core"),) * n_outs,
                  check_rep=False), keep_unused=True)

    concat_in = [np.stack([np.asarray(m[name]) for m in in_maps])
                 for name in in_names]
    dev_in = [jax.device_put(a, spec) for a in concat_in]
    jax.block_until_ready(dev_in)
    dev_zeros = [jax.device_put(np.zeros((n_cores, *z.shape), z.dtype), spec)
                 for z in zero_outs]
    jax.block_until_ready(dev_zeros)

    def one_call():
        t0 = time.perf_counter()
        outs = sharded(*dev_in, *dev_zeros)
        jax.block_until_ready(outs)
        return time.perf_counter() - t0, outs

    _, outs = one_call()            # compile + first exec
    hw_ns = None
    if os.environ.get("KERNEL_NTFF", "0") == "1":
        hw_ns, outs = _ntff_profile(one_call)
    if reps > 0:
        times = [one_call()[0] for _ in range(reps)]
        if hw_ns is None:
            print(f"HW exec time: {min(times) * 1e9:.0f} ns")
        print("warm wall (s):", [f"{t:.4f}" for t in times])
    if hw_ns is not None:
        print(f"HW exec time: {hw_ns:.0f} ns")
    return [
        {name: np.asarray(outs[i]).reshape(n_cores, *out_avals[i].shape)[c]
         for i, name in enumerate(out_names)}
        for c in range(n_cores)
    ]


def _ntff_profile(one_call, outdir="/tmp/kernel_ntff"):
    """NTFF-profile one warm exec; return (max device total_time ns, outs)."""
    import ctypes
    import shutil
    import subprocess
    try:
        lib = ctypes.CDLL("/opt/axon/libaxon_pjrt.so")
        lib.axon_start_nrt_profile.argtypes = [
            ctypes.POINTER(ctypes.c_int64), ctypes.c_size_t]
        lib.axon_start_nrt_profile.restype = ctypes.c_int64
        lib.axon_stop_nrt_profile.argtypes = [ctypes.c_char_p]
        lib.axon_stop_nrt_profile.restype = ctypes.c_int64
        shutil.rmtree(outdir, ignore_errors=True)
        os.makedirs(outdir, exist_ok=True)
        if lib.axon_start_nrt_profile(None, 0) != 0:
            raise RuntimeError("profile start failed")
        try:
            _, outs = one_call()
        finally:
            n = lib.axon_stop_nrt_profile(outdir.encode())
        if n <= 0:
            raise RuntimeError("no profile files")
        times = []
        for f in sorted(os.listdir(outdir)):
            if "jit__body" not in f or not f.endswith(".ntff"):
                continue
            neff = os.path.join(outdir, f.split("-device")[0] + ".neff")
            r = subprocess.run(
                ["neuron-profile", "view", "-n", neff,
                 "-s", os.path.join(outdir, f),
                 "--output-format", "summary-text"],
                capture_output=True, text=True, timeout=600)
            for line in r.stdout.splitlines():
                parts = line.strip().split()
                if len(parts) == 2 and parts[0] == "total_time":
                    times.append(float(parts[1]))
        if not times:
            raise RuntimeError("no total_time in profiles")
        return max(times) * 1e9, outs
    except Exception as e:  # profiling unavailable -> caller falls back
        print(f"(NTFF profiling unavailable: {e})")
        return None, one_call()[1]


if __name__ == "__main__":
    d = np.load("/tmp/inputs.npz")
    out = kernel(**{k: d[k] for k in d.files})
    ref = np.load("/tmp/ref.npy")
    err = np.abs(out - ref).max() / np.abs(ref).max()
    print("Relative error:", err)


# revision 5
# speedup vs baseline: 1.0052x; 1.0052x over previous
"""HGCN (2x hyperbolic GCN layer + MLP head) as a distributed Bass/Tile kernel
for 8 trn2 NeuronCores — ap_gather edition.

Math: logmap0(expmap0(v)) == v for this problem's value ranges, so the network
collapses to
    t2  = sigmoid(meanagg(X) @ W1 + b1)
    t3  = sigmoid(meanagg(t2) @ W2 + b2)
    out = relu(t3 @ W3 + b3) @ W4 + b4
(validated to 6e-7 against the jax reference by the previous baseline; the
numeric pipeline here reproduces the baseline's 0.93% rel err in host sim).

Distribution/layout: dst nodes sharded 8 ways (12500/core). The full node
table lives in SBUF in "octant" layout: partition 16o+j holds features
4j..4j+3 of octant-o nodes (octant o = core o's dst shard, in core o's
processing order), with row 0 zeroed. Each gpsimd Q7 core o gathers, via one
ap_gather custom op per block, the source rows of the edges whose src lies in
octant o (per-dst slot lists padded to W = max octant degree; -1 pads gather
the zero row). A strided DVE tree-sum reduces slots, 3 partition-halving adds
combine the 8 octant partials, and 4 matmuls against host-sliced W[r::4,:]
apply the layer weight while converting the packed [16,dst,4] layout to
[64,dst] — no transposes anywhere. Layer-1 outputs are folded back to packed
layout with 4 one-hot matmuls and AllGathered: the collective output IS the
layer-2 table. Weights are tiny and replicated.
"""

import os
import numpy as np
import ml_dtypes

import concourse.bass as bass
import concourse.bacc as bacc
import concourse.tile as tile
from concourse import mybir
from concourse.library_config import ap_gather as _apg_lib

NC = 8
SH = 12500
D = 64
NDCAP = 448       # dsts per block
POSCAP = 2048     # gather positions per block

BF16 = mybir.dt.bfloat16
F32 = mybir.dt.float32
I16 = mybir.dt.int16

bfloat16 = ml_dtypes.bfloat16


def _ceil16(x):
    return (x + 15) // 16 * 16


def _ceil64(x):
    # 64-position (4 idx-column) granularity: the ap_gather ucode miswrites
    # positions 16..31 of each 128-chunk when its idx AP starts at a column
    # offset = 3 (mod 4), so keep every block's idx slice 4-column aligned
    return (x + 63) // 64 * 64


def _preprocess(edge_index, n_nodes):
    """Host-side index preprocessing (layout only, no input arithmetic)."""
    src = np.asarray(edge_index[0], np.int64)
    dst = np.asarray(edge_index[1], np.int64)
    core = dst // SH
    octv = src // SH

    pm = []
    pos_of = np.empty(n_nodes, np.int64)
    for k in range(NC):
        m = core == k
        d = dst[m] - k * SH
        o = octv[m]
        s = src[m]
        deg = np.bincount(d, minlength=SH)
        dego = np.bincount(d * NC + o, minlength=SH * NC).reshape(SH, NC)
        W = np.maximum(dego.max(axis=1), 1)
        order = np.argsort(-W, kind="stable")
        pos_of[k * SH + order] = np.arange(SH)
        pm.append(dict(d=d, o=o, s=s, deg=deg, W=W, order=order))

    # uniform W envelope across cores (one compiled program for all cores)
    Wu = np.max(np.stack([p["W"][p["order"]] for p in pm]), axis=0)

    blocks = []
    p0 = 0
    i = 0
    while i < SH:
        Wv = int(Wu[i])
        j = i
        while j < SH and Wu[j] == Wv:
            j += 1
        a = i
        while a < j:
            nd = min(NDCAP, j - a, POSCAP // Wv)
            b_ = a + nd
            plen = _ceil64(nd * Wv)
            blocks.append((a, b_, Wv, p0, plen))
            p0 += plen
            a = b_
        i = j
    TOT = p0

    pos_base = np.empty(SH, np.int64)
    for (a, b_, Wv, q0, plen) in blocks:
        pos_base[a:b_] = q0 + np.arange(b_ - a) * Wv

    idxw = np.empty((NC, 128, TOT // 16), np.int16)
    dinv = np.empty((NC, 16, SH), np.float32)
    for k in range(NC):
        p = pm[k]
        d, o, s = p["d"], p["o"], p["s"]
        order = p["order"]
        inv = np.empty(SH, np.int64)
        inv[order] = np.arange(SH)
        key = d * NC + o
        ordE = np.argsort(key, kind="stable")
        ke = key[ordE]
        first = np.r_[True, ke[1:] != ke[:-1]]
        starts = np.flatnonzero(first)
        gid = np.cumsum(first) - 1
        rank = np.arange(len(ke)) - starts[gid]
        pe = pos_base[inv[d[ordE]]] + rank
        val = (pos_of[s[ordE]] + 1).astype(np.int16)
        L = np.full((NC, TOT), 0, np.int16)   # pads gather zero row 0
        L[o[ordE], pe] = val
        for oo in range(NC):
            idxw[k, oo * 16:(oo + 1) * 16, :] = L[oo].reshape(TOT // 16, 16).T
        dv = (1.0 / np.maximum(p["deg"][order], 1)).astype(np.float32)
        dinv[k] = np.broadcast_to(dv, (16, SH))
    orders = np.stack([p["order"] for p in pm])
    return dict(blocks=blocks, TOT=TOT, idxw=idxw, dinv=dinv, orders=orders)


def _build_program(blocks, TOT, dbg=False):
    nc = bacc.Bacc("TRN2", target_bir_lowering=False, debug=False,
                   enable_asserts=False, num_devices=NC)
    ADD = mybir.AluOpType.add
    MULT = mybir.AluOpType.mult

    xtab_d = nc.dram_tensor("xtab", [128, SH + 1, 4], BF16,
                            kind="ExternalInput")
    idx_d = nc.dram_tensor("idx", [128, TOT // 16], I16, kind="ExternalInput")
    dinv_d = nc.dram_tensor("dinv", [D, SH], BF16, kind="ExternalInput")
    w1_d = nc.dram_tensor("w1s", [128, 4 * D], BF16, kind="ExternalInput")
    w2_d = nc.dram_tensor("w2s", [128, 4 * D], BF16, kind="ExternalInput")
    w3_d = nc.dram_tensor("w3", [D, 128], BF16, kind="ExternalInput")
    w4_d = nc.dram_tensor("w4", [128, 40], BF16, kind="ExternalInput")
    er_d = nc.dram_tensor("er", [D, 4 * 16], BF16, kind="ExternalInput")
    b1_d = nc.dram_tensor("b1", [D, 1], F32, kind="ExternalInput")
    b2_d = nc.dram_tensor("b2", [D, 1], F32, kind="ExternalInput")
    b3_d = nc.dram_tensor("b3", [128, 1], F32, kind="ExternalInput")
    b4_d = nc.dram_tensor("b4", [40, 1], F32, kind="ExternalInput")
    outT_d = nc.dram_tensor("outT", [40, SH], F32, kind="ExternalOutput")
    t2self = nc.dram_tensor("t2self", [16, SH, 4], BF16)
    t2full = nc.dram_tensor("t2full", [128, SH, 4], BF16)
    dbg_t = {}
    if dbg:
        dbg_a = int(os.environ.get('DBG_A', str(blocks[0][0])))
        (a0, b0, W0, q0_, p0_) = [b for b in blocks if b[0] == dbg_a][0]
        nd0 = b0 - a0
        dbg_t["dG"] = nc.dram_tensor("dG", [128, p0_, 4], BF16,
                                     kind="ExternalOutput")
        dbg_t["dTb"] = nc.dram_tensor("dTb", [128, nd0, 4], BF16,
                                      kind="ExternalOutput")
        dbg_t["dAb"] = nc.dram_tensor("dAb", [128, nd0, 4], BF16,
                                      kind="ExternalOutput")
        dbg_t["dst"] = nc.dram_tensor("dst1", [D, nd0], BF16,
                                      kind="ExternalOutput")
        dbg_t["dt2b"] = nc.dram_tensor("dt2b", [16, nd0, 4], BF16,
                                       kind="ExternalOutput")
        dbg_t["dt2full"] = nc.dram_tensor("dt2full", [128, SH, 4], BF16,
                                          kind="ExternalOutput")

    from contextlib import ExitStack
    with tile.TileContext(nc) as tc, ExitStack() as es:
        const = es.enter_context(tc.tile_pool(name="const", bufs=1))
        tabp = es.enter_context(tc.tile_pool(name="tabp", bufs=1))
        gpool = es.enter_context(tc.tile_pool(name="gpool", bufs=2))
        tpool = es.enter_context(tc.tile_pool(name="tpool", bufs=1))
        spool = es.enter_context(tc.tile_pool(name="spool", bufs=2))
        psum = es.enter_context(tc.tile_pool(name="psum", bufs=2,
                                             space="PSUM"))

        nc.gpsimd.load_library(_apg_lib)
        idx_s = const.tile([128, TOT // 16], I16)
        nc.sync.dma_start(out=idx_s[:], in_=idx_d[:])
        dinv_s = const.tile([D, SH], BF16)
        nc.sync.dma_start(out=dinv_s[:], in_=dinv_d[:])
        w1_s = const.tile([128, 4 * D], BF16)
        nc.sync.dma_start(out=w1_s[:], in_=w1_d[:])
        w2_s = const.tile([128, 4 * D], BF16)
        nc.sync.dma_start(out=w2_s[:], in_=w2_d[:])
        w3_s = const.tile([D, 128], BF16)
        nc.sync.dma_start(out=w3_s[:], in_=w3_d[:])
        w4_s = const.tile([128, 40], BF16)
        nc.sync.dma_start(out=w4_s[:], in_=w4_d[:])
        er_s = const.tile([D, 4 * 16], BF16)
        nc.sync.dma_start(out=er_s[:], in_=er_d[:])
        b1_s = const.tile([D, 1], F32)
        nc.sync.dma_start(out=b1_s[:], in_=b1_d[:])
        b2_s = const.tile([D, 1], F32)
        nc.sync.dma_start(out=b2_s[:], in_=b2_d[:])
        b3_s = const.tile([128, 1], F32)
        nc.sync.dma_start(out=b3_s[:], in_=b3_d[:])
        b4_s = const.tile([40, 1], F32)
        nc.sync.dma_start(out=b4_s[:], in_=b4_d[:])

        def do_block(blk, tab, second):
            (a, b_, Wv, q0, plen) = blk
            nd = b_ - a
            dump = dbg and not second and a == int(os.environ.get('DBG_A', str(blocks[0][0])))
            G = gpool.tile([128, POSCAP, 4], BF16, tag="G")
            nc.gpsimd.ap_gather(G[:, :plen, :], tab[:],
                                idx_s[:, q0 // 16:(q0 + plen) // 16],
                                channels=128, num_elems=SH + 1, d=4,
                                num_idxs=plen)
            g4 = G[:, :nd * Wv, :].rearrange("p (n w) f -> p n w f", w=Wv)
            # per-octant slot reduction -> Tb [128, nd, 4] bf16 (one bf16
            # rounding of the f32-exact partial; octant combine happens in
            # the PE's f32 accumulation via the 8x-replicated W slices)
            if Wv == 1:
                red = g4[:, :, 0, :]
            elif Wv == 2:
                Tb = spool.tile([128, NDCAP, 4], BF16, tag="Tb")
                nc.vector.tensor_tensor(out=Tb[:, :nd, :],
                                        in0=g4[:, :, 0, :],
                                        in1=g4[:, :, 1, :], op=ADD)
                red = Tb[:, :nd, :]
            else:
                h = Wv // 2
                ns = h + (Wv % 2)
                T = tpool.tile([128, 1280, 4], F32, tag="T")
                assert nd * ns <= 1280
                t4 = T[:, :nd * ns, :].rearrange("p (n w) f -> p n w f", w=ns)
                nc.vector.tensor_tensor(out=t4[:, :, 0:h, :],
                                        in0=g4[:, :, 0:h, :],
                                        in1=g4[:, :, h:2 * h, :], op=ADD)
                if Wv % 2:
                    nc.vector.tensor_copy(out=t4[:, :, h, :],
                                          in_=g4[:, :, 2 * h, :])
                n = ns
                while n > 2:
                    hh = n // 2
                    nc.vector.tensor_tensor(out=t4[:, :, 0:hh, :],
                                            in0=t4[:, :, 0:hh, :],
                                            in1=t4[:, :, hh:2 * hh, :],
                                            op=ADD)
                    if n % 2:
                        nc.vector.tensor_copy(out=t4[:, :, hh, :],
                                              in_=t4[:, :, n - 1, :])
                    n = hh + (n % 2)
                Tb = spool.tile([128, NDCAP, 4], BF16, tag="Tb")
                nc.vector.tensor_tensor(out=Tb[:, :nd, :],
                                        in0=t4[:, :, 0, :],
                                        in1=t4[:, :, 1, :], op=ADD)
                red = Tb[:, :nd, :]
            ws = w2_s if second else w1_s
            bs = b2_s if second else b1_s
            pmm = psum.tile([D, NDCAP], F32, tag="pm", space="PSUM")
            for r in range(4):
                nc.tensor.matmul(pmm[:, :nd], lhsT=ws[:, r * D:(r + 1) * D],
                                 rhs=red[:, :, r], start=(r == 0),
                                 stop=(r == 3))
            # mean normalization applied post-matmul (linear), one [64,nd] op
            sp = spool.tile([D, NDCAP], F32, tag="sp")
            nc.vector.tensor_tensor(out=sp[:, :nd], in0=pmm[:, :nd],
                                    in1=dinv_s[:, a:b_], op=MULT)
            st = spool.tile([D, NDCAP], BF16, tag="st")
            nc.scalar.activation(st[:, :nd], sp[:, :nd],
                                 mybir.ActivationFunctionType.Sigmoid,
                                 bias=bs[:, :1])
            return (blk, st)

        def do_back(pend, second):
            (blk, st) = pend
            (a, b_, Wv, q0, plen) = blk
            nd = b_ - a
            dump = False
            if not second:
                t2b = spool.tile([16, NDCAP, 4], BF16, tag="t2b")
                for r in range(4):
                    pf = psum.tile([16, NDCAP], F32, tag="pf", space="PSUM")
                    nc.tensor.matmul(pf[:, :nd],
                                     lhsT=er_s[:, r * 16:(r + 1) * 16],
                                     rhs=st[:, :nd], start=True, stop=True)
                    nc.vector.tensor_copy(out=t2b[:, :nd, r], in_=pf[:, :nd])
                nc.sync.dma_start(out=t2self[:, a:b_, :], in_=t2b[:, :nd, :])
            else:
                p3 = psum.tile([128, NDCAP], F32, tag="p3", space="PSUM")
                nc.tensor.matmul(p3[:, :nd], lhsT=w3_s[:], rhs=st[:, :nd],
                                 start=True, stop=True)
                hb = spool.tile([128, NDCAP], BF16, tag="hb")
                nc.scalar.activation(hb[:, :nd], p3[:, :nd],
                                     mybir.ActivationFunctionType.Relu,
                                     bias=b3_s[:, :1])
                p4 = psum.tile([40, NDCAP], F32, tag="p4", space="PSUM")
                nc.tensor.matmul(p4[:, :nd], lhsT=w4_s[:], rhs=hb[:, :nd],
                                 start=True, stop=True)
                o4 = spool.tile([40, NDCAP], F32, tag="o4")
                nc.vector.tensor_scalar_add(o4[:, :nd], p4[:, :nd],
                                            b4_s[:, :1])
                nc.sync.dma_start(out=outT_d[:, a:b_], in_=o4[:, :nd])

        tab = tabp.tile([128, SH + 1, 4], BF16, tag="tab")
        nc.sync.dma_start(out=tab[:], in_=xtab_d[:])
        if dbg and os.environ.get("DBG_MODE", "") == "gatheronly":
            dga = nc.dram_tensor("dGall", [128, TOT, 4], BF16,
                                 kind="ExternalOutput")
            gopool = es.enter_context(tc.tile_pool(
                name="gopool", bufs=int(os.environ.get("GONLY_BUFS", "2"))))
            for (a, b_, Wv, q0, plen) in blocks:
                G = gopool.tile([128, POSCAP, 4], BF16, tag="G")
                nc.gpsimd.ap_gather(G[:, :plen, :], tab[:],
                                    idx_s[:, q0 // 16:(q0 + plen) // 16],
                                    channels=128, num_elems=SH + 1, d=4,
                                    num_idxs=plen)
                nc.sync.dma_start(out=dga[:, q0:q0 + plen, :],
                                  in_=G[:, :plen, :])
            zt = const.tile([40, SH], F32)
            nc.vector.memset(zt[:], 0.0)
            nc.sync.dma_start(out=outT_d[:], in_=zt[:])
        else:
            pend = None
            for blk in blocks:
                new = do_block(blk, tab, second=False)
                if pend is not None:
                    do_back(pend, False)
                pend = new
            do_back(pend, False)
            nc.gpsimd.collective_compute(
                "AllGather",
                mybir.AluOpType.bypass,
                replica_groups=[list(range(NC))],
                ins=[t2self.ap().opt()],
                outs=[t2full.ap().opt()],
            )
            if dbg:
                nc.sync.dma_start(out=dbg_t["dt2full"][:], in_=t2full[:])
            tab2 = tabp.tile([128, SH + 1, 4], BF16, tag="tab")
            nc.sync.dma_start(out=tab2[:, 1:SH + 1, :], in_=t2full[:])
            nc.vector.memset(tab2[:, 0:1, :], 0.0)
            pend = None
            for blk in blocks:
                new = do_block(blk, tab2, second=True)
                if pend is not None:
                    do_back(pend, True)
                pend = new
            do_back(pend, True)

    nc.compile()
    return nc


def _host_inputs(inputs, meta):
    X = np.asarray(inputs["features"], np.float32)
    orders = meta["orders"]
    xtab = np.zeros((128, SH + 1, 4), bfloat16)
    for o in range(NC):
        arr = X[o * SH + orders[o]].astype(bfloat16)       # [SH, 64]
        xtab[16 * o:16 * (o + 1), 1:, :] = (
            arr.reshape(SH, 16, 4).transpose(1, 0, 2))

    def wslices(Wm):
        Wb = np.asarray(Wm, np.float32).astype(bfloat16)
        s = np.concatenate([Wb[r::4, :] for r in range(4)], axis=1)
        return np.tile(s, (8, 1))              # replicate across octant groups

    er = np.zeros((D, 64), bfloat16)
    for j in range(16):
        for r in range(4):
            er[4 * j + r, r * 16 + j] = 1

    common = dict(
        xtab=xtab,
        w1s=wslices(inputs["W1"]),
        w2s=wslices(inputs["W2"]),
        w3=np.asarray(inputs["W3"], np.float32).astype(bfloat16),
        w4=np.asarray(inputs["W4"], np.float32).astype(bfloat16),
        er=er,
        b1=np.asarray(inputs["b1"], np.float32).reshape(D, 1),
        b2=np.asarray(inputs["b2"], np.float32).reshape(D, 1),
        b3=np.asarray(inputs["b3"], np.float32).reshape(128, 1),
        b4=np.asarray(inputs["b4"], np.float32).reshape(40, 1),
    )
    return [dict(common, idx=meta["idxw"][k],
                 dinv=np.broadcast_to(meta["dinv"][k][0], (D, SH))
                 .astype(bfloat16)) for k in range(NC)]


def kernel(features, edge_index, W1, b1, W2, b2, W3, b3, W4, b4):
    n_nodes = features.shape[0]
    inputs = dict(features=features, edge_index=edge_index, W1=W1, b1=b1,
                  W2=W2, b2=b2, W3=W3, b3=b3, W4=W4, b4=b4)
    meta = _preprocess(edge_index, n_nodes)
    nc = _build_program(meta["blocks"], meta["TOT"])
    in_maps = _host_inputs(inputs, meta)
    results = _run_spmd_timed(nc, in_maps,
                              reps=int(os.environ.get("KERNEL_REPS", "0")))
    out = np.empty((n_nodes, 40), np.float32)
    for k in range(NC):
        outT = np.asarray(results[k]["outT"], np.float32)
        out[k * SH + meta["orders"][k]] = outT.T
    return out


def _run_spmd_timed(nc, in_maps, reps=0):
    """bass2jax.run_bass_via_pjrt's multi-core path with inputs device_put
    once (sharded), optional repeated timed executions, and optional NTFF
    profiling (KERNEL_NTFF=1) for the true on-device execution time."""
    import time
    import jax
    from jax.sharding import Mesh, PartitionSpec, NamedSharding
    from jax.experimental.shard_map import shard_map
    from concourse import bass2jax, mybir as mb

    bass2jax.install_neuronx_cc_hook()
    n_cores = len(in_maps)
    partition_name = (nc.partition_id_tensor.name
                      if nc.partition_id_tensor else None)
    in_names, out_names, out_avals, zero_outs = [], [], [], []
    for alloc in nc.m.functions[0].allocations:
        if not isinstance(alloc, mb.MemoryLocationSet):
            continue
        name = alloc.memorylocations[0].name
        if alloc.kind == "ExternalInput":
            if name != partition_name:
                in_names.append(name)
        elif alloc.kind == "ExternalOutput":
            shape = tuple(alloc.tensor_shape)
            dtype = mb.dt.np(alloc.dtype)
            out_avals.append(jax.core.ShapedArray(shape, dtype))
            zero_outs.append(np.zeros(shape, dtype))
            out_names.append(name)
    n_params = len(in_names)
    n_outs = len(out_avals)
    all_in_names = list(in_names) + list(out_names)
    if partition_name is not None:
        all_in_names.append(partition_name)

    def _body(*args):
        operands = list(args)
        if partition_name is not None:
            operands.append(bass2jax.partition_id_tensor())
        return tuple(bass2jax._bass_exec_p.bind(
            *operands, out_avals=tuple(out_avals),
            in_names=tuple(all_in_names), out_names=tuple(out_names),
            lowering_input_output_aliases=(),
            sim_require_finite=True, sim_require_nnan=True, nc=nc))

    devices = jax.devices()[:n_cores]
    mesh = Mesh(np.asarray(devices), ("core",))
    spec = NamedSharding(mesh, PartitionSpec("core"))
    sharded = jax.jit(
        shard_map(_body, mesh=mesh,
                  in_specs=(PartitionSpec("core"),) * (n_params + n_outs),
                  out_specs=(PartitionSpec("core"),) * n_outs,
                  check_rep=False), keep_unused=True)

    concat_in = [np.stack([np.asarray(m[name]) for m in in_maps])
                 for name in in_names]
    dev_in = [jax.device_put(a, spec) for a in concat_in]
    jax.block_until_ready(dev_in)
    dev_zeros = [jax.device_put(np.zeros((n_cores, *z.shape), z.dtype), spec)
                 for z in zero_outs]
    jax.block_until_ready(dev_zeros)

    def one_call():
        t0 = time.perf_counter()
        outs = sharded(*dev_in, *dev_zeros)
        jax.block_until_ready(outs)
        return time.perf_counter() - t0, outs

    _, outs = one_call()            # compile + first exec
    hw_ns = None
    if os.environ.get("KERNEL_NTFF", "0") == "1":
        hw_ns, outs = _ntff_profile(one_call)
    if reps > 0:
        times = [one_call()[0] for _ in range(reps)]
        if hw_ns is None:
            print(f"HW exec time: {min(times) * 1e9:.0f} ns")
        print("warm wall (s):", [f"{t:.4f}" for t in times])
    if hw_ns is not None:
        print(f"HW exec time: {hw_ns:.0f} ns")
    return [
        {name: np.asarray(outs[i]).reshape(n_cores, *out_avals[i].shape)[c]
         for i, name in enumerate(out_names)}
        for c in range(n_cores)
    ]


def _ntff_profile(one_call, outdir="/tmp/kernel_ntff"):
    """NTFF-profile one warm exec; return (max device total_time ns, outs)."""
    import ctypes
    import shutil
    import subprocess
    try:
        lib = ctypes.CDLL("/opt/axon/libaxon_pjrt.so")
        lib.axon_start_nrt_profile.argtypes = [
            ctypes.POINTER(ctypes.c_int64), ctypes.c_size_t]
        lib.axon_start_nrt_profile.restype = ctypes.c_int64
        lib.axon_stop_nrt_profile.argtypes = [ctypes.c_char_p]
        lib.axon_stop_nrt_profile.restype = ctypes.c_int64
        shutil.rmtree(outdir, ignore_errors=True)
        os.makedirs(outdir, exist_ok=True)
        if lib.axon_start_nrt_profile(None, 0) != 0:
            raise RuntimeError("profile start failed")
        try:
            _, outs = one_call()
        finally:
            n = lib.axon_stop_nrt_profile(outdir.encode())
        if n <= 0:
            raise RuntimeError("no profile files")
        times = []
        for f in sorted(os.listdir(outdir)):
            if "jit__body" not in f or not f.endswith(".ntff"):
                continue
            neff = os.path.join(outdir, f.split("-device")[0] + ".neff")
            r = subprocess.run(
                ["neuron-profile", "view", "-n", neff,
                 "-s", os.path.join(outdir, f),
                 "--output-format", "summary-text"],
                capture_output=True, text=True, timeout=600)
            for line in r.stdout.splitlines():
                parts = line.strip().split()
                if len(parts) == 2 and parts[0] == "total_time":
                    times.append(float(parts[1]))
        if not times:
            raise RuntimeError("no total_time in profiles")
        return max(times) * 1e9, outs
    except Exception as e:  # profiling unavailable -> caller falls back
        print(f"(NTFF profiling unavailable: {e})")
        return None, one_call()[1]


if __name__ == "__main__":
    d = np.load("/tmp/inputs.npz")
    out = kernel(**{k: d[k] for k in d.files})
    ref = np.load("/tmp/ref.npy")
    err = np.abs(out - ref).max() / np.abs(ref).max()
    print("Relative error:", err)
